# revision 21
# baseline (speedup 1.0000x reference)
"""Trainium2 Bass kernel for nn_A2CDense (dense GNN message-passing block).

Data-parallel over the graph-batch dim B=64: 8 graphs per NeuronCore, 8 cores.

Structure (per core, 8 graphs):
- The big one-hot incidence operands stream from HBM in fp16 (one-hot
  matrices are exact in fp16): E_s in [v,e] layout, E_r in BOTH [v,e] and
  [e,v] layouts (so the scatter-aggregate needs no on-chip transpose of the
  2048-wide matrix).
- Edge-MLP layer 1 uses associativity through the one-hot gathers:
    h1 = W1_Ea @ E_a + (W1_snd @ V_a) @ E_s + (W1_rcv @ V_a) @ E_r
  with the broadcast global-feature term folded into the activation bias.
  The four 512-edge tiles of a graph are STACKED across the 4 PE
  partition-quadrants via tile_position column-packing, so the four tiles'
  matmuls run concurrently on the 16 32x32 sub-arrays and each MLP layer
  needs ONE 128-partition gelu instead of four.
- Layers 2-3 run diagonal-packed (row=col=32j); layer 4 row-pack-unstacks
  back to flat [32, 2048] so out_E, its transposes (for the aggregate), and
  the per-graph reductions stay simple.
- agg = out_E @ E_r^T accumulates column-packed into a stacked [128, 512]
  PSUM (chunk k -> quadrant k%4); the node-MLP layer-1 aggregate term uses
  4x-replicated weights so the quadrant partial sums collapse in the same
  matmul. Node MLPs of 4 graphs run stacked across quadrants as one wave.
- PE matmul dtype is float32r (tf32-like, full-rate) for the small precise
  paths (M_s/M_r, folds, global MLP, out_E transposes); fp16 for the big
  streamed paths. PSUM accumulation is always fp32.
"""
import os
import numpy as np


def _patch_ldw_opt():
    # walrus's LDWEIGHTS dedup pass is disabled by default in this bass
    # pipeline; our kernel re-loads identical 32-col stationaries many times
    # per graph, so flip it on (validated by the rel-err check).
    from concourse import bass_utils as _bu
    if getattr(_bu, "_ldw_patched", False):
        return
    _orig = _bu.run_command

    def _run(argv, **kw):
        argv = [a.replace("--enable-ldw-opt=false", "--enable-ldw-opt=true")
                if isinstance(a, str) else a for a in argv]
        return _orig(argv, **kw)

    _bu.run_command = _run
    _bu._ldw_patched = True


if os.environ.get("A2C_LDW_OPT", "0") == "1":
    _patch_ldw_opt()

B, NV, NE = 64, 512, 2048
DF = 32
N_CORES = 8
GPC = B // N_CORES  # graphs per core
ET = 4              # edge tiles per graph
ETW = NE // ET      # 512
VC = NV // 128      # 4 v-chunks
EC = NE // 128      # 16 e-chunks
NW = GPC // 4       # node waves per core

_cached = {}

# Wk (f32r) indices
FE_U, FV_U, FU_U, FU_VS, FU_ES, FU_W2, FU_W3, FU_W4 = range(8)
# Wf16 indices
F16_EA, F16_VA, F16_VU = 0, 1, 2
# Wrep16 indices (all 4x vertically replicated)
R_FEW2, R_FEW3, R_FEW4, R_FVW2, R_FVW3, R_FVW4, R_AGG = range(7)


def _build_nc():
    import concourse.bass as bass
    import concourse.bacc as bacc
    import concourse.tile as tile
    import concourse.mybir as mybir
    from concourse.masks import make_identity

    f32 = mybir.dt.float32
    f32r = mybir.dt.float32r
    fp16 = mybir.dt.float16
    fp8 = mybir.dt.float8e4
    AF = mybir.ActivationFunctionType

    nc = bacc.Bacc("TRN2", target_bir_lowering=False, debug=False,
                   num_devices=N_CORES)

    E_s16 = nc.dram_tensor("E_s16", [GPC, 128, VC, NE], fp16, kind="ExternalInput")
    E_r16 = nc.dram_tensor("E_r16", [GPC, 128, VC, NE], fp16, kind="ExternalInput")
    E_rT8 = nc.dram_tensor("E_rT8", [GPC, 128, EC, NV], fp8,
                           kind="ExternalInput")
    E_a16 = nc.dram_tensor("E_a16", [GPC, DF, NE], fp16, kind="ExternalInput")
    V_a = nc.dram_tensor("V_a", [GPC, DF, NV], f32r, kind="ExternalInput")
    V_a16 = nc.dram_tensor("V_a16", [GPC, DF, NV], fp16, kind="ExternalInput")
    uT = nc.dram_tensor("uT", [DF, GPC], f32r, kind="ExternalInput")
    u16 = nc.dram_tensor("u16", [DF, GPC], fp16, kind="ExternalInput")
    Wk = nc.dram_tensor("Wk", [DF, 8, DF], f32r, kind="ExternalInput")
    Wsr = nc.dram_tensor("Wsr", [DF, 2 * DF], f32r, kind="ExternalInput")
    Wfold = nc.dram_tensor("Wfold", [DF, 128], f32r, kind="ExternalInput")
    Wf16 = nc.dram_tensor("Wf16", [DF, 3, DF], fp16, kind="ExternalInput")
    Wrep16 = nc.dram_tensor("Wrep16", [128, 7, DF], fp16,
                            kind="ExternalInput")
    BIr = nc.dram_tensor("BIr", [128, 12], f32, kind="ExternalInput")

    out_E = nc.dram_tensor("out_E", [GPC, DF, NE], f32r, kind="ExternalOutput")
    out_V = nc.dram_tensor("out_V", [GPC, DF, NV], f32r, kind="ExternalOutput")
    out_uT = nc.dram_tensor("out_uT", [DF, GPC], f32r, kind="ExternalOutput")

    with tile.TileContext(nc) as tc:
        with (
            tc.tile_pool(name="consts", bufs=1) as consts,
            tc.tile_pool(name="work", bufs=1) as work,
        ):
            W = consts.tile([DF, 8, DF], f32r, tag="W")
            Wsr_t = consts.tile([DF, 2 * DF], f32r, tag="Wsr")
            Wfold_t = consts.tile([DF, 128], f32r, tag="Wfold")
            Wf = consts.tile([DF, 3, DF], fp16, tag="Wf")
            Wr = consts.tile([128, 7, DF], fp16, tag="Wr")
            BIt = consts.tile([128, 12], f32, tag="BI")
            u_all = consts.tile([DF, GPC], f32r, tag="u_all")
            u16_t = consts.tile([DF, GPC], fp16, tag="u16")
            ident = consts.tile([128, 128], f32, tag="ident")
            ident_r = consts.tile([128, 128], f32r, tag="ident_r")
            for t, src in ((W, Wk), (Wsr_t, Wsr), (Wfold_t, Wfold), (Wf, Wf16),
                           (Wr, Wrep16), (BIt, BIr), (u_all, uT), (u16_t, u16)):
                nc.sync.dma_start(t[:], src.ap())
            make_identity(nc, ident[:])
            nc.vector.tensor_copy(ident_r[:], ident[:])

            # folded L1 biases
            b1e_st = work.tile([128, GPC], f32, tag="b1e")   # per graph
            b1v_st = work.tile([128, NW], f32, tag="b1v")    # per node wave
            with tc.tile_pool(name="pfold", bufs=1, space="PSUM") as pfold:
                ps_be = pfold.tile([128, GPC], f32, tag="pbe")
                nc.tensor.matmul(ps_be[:], Wfold_t[:], u_all[:],
                                 start=True, stop=True)
                nc.vector.tensor_scalar_add(b1e_st[:], ps_be[:], BIt[:, 0:1])
                ps_bv = pfold.tile([128, NW], f32, tag="pbv")
                for gi in range(4):
                    nc.tensor.matmul(
                        ps_bv[32 * gi:32 * (gi + 1), :], Wf[:, F16_VU, :],
                        u16_t[:].rearrange("d (w gi) -> d w gi", gi=4)
                        [:, :, gi],
                        start=True, stop=True, tile_position=(0, 32 * gi),
                        skip_group_check=True)
                nc.vector.tensor_scalar_add(b1v_st[:], ps_bv[:], BIt[:, 4:5])

            oVsum = work.tile([DF, GPC], f32, tag="oVsum")
            oEsum = work.tile([DF, GPC], f32, tag="oEsum")

            with (
                tc.tile_pool(name="big", bufs=2) as big,
                tc.tile_pool(name="big3", bufs=3) as big3,
                tc.tile_pool(name="med", bufs=2) as med,
                tc.tile_pool(name="node", bufs=6) as nodep,
                tc.tile_pool(name="small", bufs=3) as small,
                tc.tile_pool(name="pst", bufs=1, space="PSUM") as pstp,
                tc.tile_pool(name="pagg", bufs=1, space="PSUM") as paggp,
                tc.tile_pool(name="ph4", bufs=1, space="PSUM") as ph4p,
                tc.tile_pool(name="poet", bufs=1, space="PSUM") as poetp,
                tc.tile_pool(name="pwarm", bufs=1, space="PSUM") as pwarmp,
            ):
                lanes = [None] * 4  # (g, va16_t, agg16_t) per node lane

                def warm():
                    pw = pwarmp.tile([DF, 2 * DF], f32, tag="pwarm")
                    nc.tensor.matmul(pw[:], ident_r[0:DF, 0:DF],
                                     ident_r[0:DF, 0:2 * DF],
                                     start=True, stop=True,
                                     skip_group_check=True)

                def emit_node_wave(w):
                    pvn = pstp.tile([128, NV], f32, tag="pst")
                    for gi in range(4):
                        nc.tensor.matmul(
                            pvn[32 * gi:32 * (gi + 1), :], Wf[:, F16_VA, :],
                            lanes[gi][1][:], start=True, stop=False,
                            tile_position=(0, 32 * gi), skip_group_check=True)
                    for gi in range(4):
                        nc.tensor.matmul(
                            pvn[32 * gi:32 * (gi + 1), :], Wr[:, R_AGG, :],
                            lanes[gi][2][:], start=False, stop=True,
                            tile_position=(0, 32 * gi), skip_group_check=True)
                    hn = small.tile([128, NV], fp16, tag="hn")
                    nc.scalar.activation(hn[:], pvn[:], AF.Gelu,
                                         bias=b1v_st[:, w:w + 1])
                    warm()
                    for wi, bi in ((R_FVW2, 5), (R_FVW3, 6)):
                        pvn2 = pstp.tile([128, NV], f32, tag="pst")
                        for gi in range(4):
                            nc.tensor.matmul(
                                pvn2[32 * gi:32 * (gi + 1), :],
                                Wr[32 * gi:32 * (gi + 1), wi, :],
                                hn[32 * gi:32 * (gi + 1), :],
                                start=True, stop=True,
                                tile_position=(32 * gi, 32 * gi),
                                skip_group_check=True)
                        hn = small.tile([128, NV], fp16, tag="hn")
                        nc.scalar.activation(hn[:], pvn2[:], AF.Gelu,
                                             bias=BIt[:, bi:bi + 1])
                        warm()
                    pn4 = ph4p.tile([DF, 4, NV], f32, tag="ph4")
                    for gi in range(4):
                        nc.tensor.matmul(
                            pn4[:, gi, :], Wr[32 * gi:32 * (gi + 1), R_FVW4, :],
                            hn[32 * gi:32 * (gi + 1), :],
                            start=True, stop=True,
                            tile_position=(32 * gi, 0), skip_group_check=True)
                    for gi in range(4):
                        g = lanes[gi][0]
                        ov_t = nodep.tile([DF, NV], f32r, tag="ov")
                        nc.scalar.activation(ov_t[:], pn4[:, gi, :], AF.Gelu,
                                             bias=BIt[0:DF, 7:8],
                                             accum_out=oVsum[:, g:g + 1])
                        nc.sync.dma_start(out_V.ap()[g], ov_t[:])

                for g in range(GPC):
                    va_t = med.tile([DF, NV], f32r, tag="va")
                    va16_t = nodep.tile([DF, NV], fp16, tag="va16")
                    ea_t = med.tile([DF, NE], fp16, tag="ea")
                    nc.sync.dma_start(va_t[:], V_a.ap()[g])
                    nc.sync.dma_start(va16_t[:], V_a16.ap()[g])
                    nc.sync.dma_start(ea_t[:], E_a16.ap()[g])
                    es_t = big.tile([128, VC, NE], fp16, tag="es")
                    for hh in range(2):
                        nc.sync.dma_start(
                            es_t[:, :, hh * (NE // 2):(hh + 1) * (NE // 2)],
                            E_s16.ap()[g][:, :, hh * (NE // 2):
                                          (hh + 1) * (NE // 2)])
                    er_t = big.tile([128, VC, NE], fp16, tag="er")
                    for hh in range(2):
                        nc.sync.dma_start(
                            er_t[:, :, hh * (NE // 2):(hh + 1) * (NE // 2)],
                            E_r16.ap()[g][:, :, hh * (NE // 2):
                                          (hh + 1) * (NE // 2)])
                    ert_t = big3.tile([128, EC, NV], fp8, tag="ert")
                    for hh in range(2):
                        nc.gpsimd.dma_start(
                            ert_t[:, hh * (EC // 2):(hh + 1) * (EC // 2), :],
                            E_rT8.ap()[g][:, hh * (EC // 2):
                                          (hh + 1) * (EC // 2), :])

                    # M_s^T / M_r^T directly: [128v, c, (Ms|Mr)]
                    ps_m = paggp.tile([128, VC, 4 * DF], f32, tag="pagg")
                    for c in range(VC):
                        nc.tensor.matmul(ps_m[:, c, 0:2 * DF],
                                         va_t[:, c * 128:(c + 1) * 128],
                                         Wsr_t[:], start=True, stop=True)
                    msrT = med.tile([128, VC, 2 * DF], fp16, tag="msrT")
                    nc.vector.tensor_copy(msrT[:], ps_m[:, :, 0:2 * DF])

                    # --- edge L1, col-packed: quadrant j = e-tile j ---
                    pst = pstp.tile([128, ETW], f32, tag="pst")
                    for j in range(ET):
                        nc.tensor.matmul(
                            pst[32 * j:32 * (j + 1), :], Wf[:, F16_EA, :],
                            ea_t[:, j * ETW:(j + 1) * ETW],
                            start=True, stop=False, tile_position=(0, 32 * j),
                            skip_group_check=True)
                    for c in range(VC):
                        for j in range(ET):
                            nc.tensor.matmul(
                                pst[32 * j:32 * (j + 1), :], msrT[:, c, 0:DF],
                                es_t[:, c, j * ETW:(j + 1) * ETW],
                                start=False, stop=False,
                                tile_position=(0, 32 * j),
                                skip_group_check=True)
                    for c in range(VC):
                        for j in range(ET):
                            nc.tensor.matmul(
                                pst[32 * j:32 * (j + 1), :],
                                msrT[:, c, DF:2 * DF],
                                er_t[:, c, j * ETW:(j + 1) * ETW],
                                start=False, stop=(c == VC - 1),
                                tile_position=(0, 32 * j),
                                skip_group_check=True)
                    h = small.tile([128, ETW], fp16, tag="h")
                    nc.scalar.activation(h[:], pst[:], AF.Gelu,
                                         bias=b1e_st[:, g:g + 1])
                    warm()
                    # L2, L3 diagonal-packed
                    for wi, bi in ((R_FEW2, 1), (R_FEW3, 2)):
                        pst2 = pstp.tile([128, ETW], f32, tag="pst")
                        for j in range(ET):
                            nc.tensor.matmul(
                                pst2[32 * j:32 * (j + 1), :],
                                Wr[32 * j:32 * (j + 1), wi, :],
                                h[32 * j:32 * (j + 1), :],
                                start=True, stop=True,
                                tile_position=(32 * j, 32 * j),
                                skip_group_check=True)
                        h = small.tile([128, ETW], fp16, tag="h")
                        nc.scalar.activation(h[:], pst2[:], AF.Gelu,
                                             bias=BIt[:, bi:bi + 1])
                        warm()
                    # L4 row-packed unstack -> flat
                    ph4 = ph4p.tile([DF, ET, ETW], f32, tag="ph4")
                    for j in range(ET):
                        nc.tensor.matmul(
                            ph4[:, j, :], Wr[32 * j:32 * (j + 1), R_FEW4, :],
                            h[32 * j:32 * (j + 1), :], start=True, stop=True,
                            tile_position=(32 * j, 0), skip_group_check=True)
                    oe_t = med.tile([DF, NE], f32r, tag="oe")
                    oeacc = small.tile([DF, ET], f32, tag="oeacc")
                    for et in range(ET):
                        nc.scalar.activation(
                            oe_t[:, et * ETW:(et + 1) * ETW], ph4[:, et, :],
                            AF.Gelu, bias=BIt[0:DF, 3:4],
                            accum_out=oeacc[:, et:et + 1])

                    # transposes of out_E + col-packed aggregate
                    pagg = paggp.tile([128, NV], f32, tag="pagg")
                    for et in range(ET):
                        ps_oet = poetp.tile([128, VC, DF], f32r, tag="poet")
                        for c in range(VC):
                            nc.tensor.transpose(
                                ps_oet[:, c, :],
                                oe_t[:, et * ETW + c * 128:
                                     et * ETW + (c + 1) * 128],
                                ident_r[0:DF, 0:DF])
                        oeT = small.tile([128, VC, DF], fp8, tag="oeT")
                        oeTl = small.tile([128, VC, DF], fp8, tag="oeTl")
                        nc.vector.tensor_copy(oeT[:], ps_oet[:])
                        nc.vector.tensor_sub(oeTl[:], ps_oet[:], oeT[:])
                        warm()
                        for c in range(VC):
                            k = et * VC + c
                            j = k % 4
                            nc.tensor.matmul(
                                pagg[32 * j:32 * (j + 1), :], oeT[:, c, :],
                                ert_t[:, k, :],
                                start=(k < 4), stop=False,
                                tile_position=(0, 32 * j),
                                skip_group_check=True)
                            nc.tensor.matmul(
                                pagg[32 * j:32 * (j + 1), :], oeTl[:, c, :],
                                ert_t[:, k, :],
                                start=False, stop=(k >= EC - 4),
                                tile_position=(0, 32 * j),
                                skip_group_check=True)

                    nc.sync.dma_start(out_E.ap()[g], oe_t[:])
                    nc.vector.reduce_sum(oEsum[:, g:g + 1], oeacc[:],
                                         axis=mybir.AxisListType.X)
                    agg16 = nodep.tile([128, NV], fp16, tag="agg16")
                    nc.vector.tensor_copy(agg16[:], pagg[:])
                    lanes[g % 4] = (g, va16_t, agg16)
                    if g % 4 == 3:
                        emit_node_wave(g // 4)

                # --- global MLP: P = [u; sumV; sumE] ---
                vs_r = work.tile([DF, GPC], f32r, tag="vs_r")
                es_r = work.tile([DF, GPC], f32r, tag="es_r")
                nc.vector.tensor_copy(vs_r[:], oVsum[:])
                nc.vector.tensor_copy(es_r[:], oEsum[:])
                phu = pstp.tile([DF, GPC], f32, tag="pst")
                nc.tensor.matmul(phu[:], W[:, FU_U, :], u_all[:],
                                 start=True, stop=False)
                nc.tensor.matmul(phu[:], W[:, FU_VS, :], vs_r[:],
                                 start=False, stop=False)
                nc.tensor.matmul(phu[:], W[:, FU_ES, :], es_r[:],
                                 start=False, stop=True)
                hu = work.tile([DF, GPC], f32r, tag="hu")
                nc.scalar.activation(hu[:], phu[:], AF.Gelu,
                                     bias=BIt[0:DF, 8:9])
                for wi, bi in ((FU_W2, 9), (FU_W3, 10)):
                    phu2 = pstp.tile([DF, GPC], f32, tag="pst")
                    nc.tensor.matmul(phu2[:], W[:, wi, :], hu[:],
                                     start=True, stop=True)
                    hu = work.tile([DF, GPC], f32r, tag="hu")
                    nc.scalar.activation(hu[:], phu2[:], AF.Gelu,
                                         bias=BIt[0:DF, bi:bi + 1])
                phu4 = pstp.tile([DF, GPC], f32, tag="pst")
                nc.tensor.matmul(phu4[:], W[:, FU_W4, :], hu[:],
                                 start=True, stop=True)
                ou_t = work.tile([DF, GPC], f32r, tag="ou")
                nc.scalar.activation(ou_t[:], phu4[:], AF.Gelu,
                                     bias=BIt[0:DF, 11:12])
                nc.sync.dma_start(out_uT.ap(), ou_t[:])

    nc.compile()
    return nc


def _get_nc():
    if "nc" not in _cached:
        _cached["nc"] = _build_nc()
    return _cached["nc"]


def _pack_host(fe_Ws, fe_bs, fv_Ws, fv_bs, fu_Ws, fu_bs):
    import ml_dtypes
    f = np.float32
    hp = ml_dtypes.bfloat16  # placeholder, replaced below
    hp = np.float16
    fe_Ws = [np.asarray(w, f) for w in fe_Ws]
    fv_Ws = [np.asarray(w, f) for w in fv_Ws]
    fu_Ws = [np.asarray(w, f) for w in fu_Ws]
    W1e, W1v, W1u = fe_Ws[0], fv_Ws[0], fu_Ws[0]
    # f32r pack
    wk = [W1e[:, 96:128].T, W1v[:, 64:96].T,
          W1u[:, 0:32].T, W1u[:, 32:64].T, W1u[:, 64:96].T,
          fu_Ws[1].T, fu_Ws[2].T, fu_Ws[3].T]
    Wk = np.ascontiguousarray(np.stack(wk, axis=1), dtype=f)      # [32, 8, 32]
    Wsr = np.ascontiguousarray(
        np.concatenate([W1e[:, 32:64].T, W1e[:, 64:96].T], axis=1), dtype=f)
    Wfold = np.ascontiguousarray(np.tile(W1e[:, 96:128].T, (1, 4)), dtype=f)
    wf = [W1e[:, 0:32].T, W1v[:, 0:32].T, W1v[:, 64:96].T]
    Wf16 = np.ascontiguousarray(np.stack(wf, axis=1), dtype=hp)   # [32, 3, 32]
    wr = [fe_Ws[1].T, fe_Ws[2].T, fe_Ws[3].T,
          fv_Ws[1].T, fv_Ws[2].T, fv_Ws[3].T, W1v[:, 32:64].T]
    Wrep16 = np.ascontiguousarray(
        np.stack([np.tile(m, (4, 1)) for m in wr], axis=1), dtype=hp)
    bs = [np.asarray(b, f) for b in (*fe_bs, *fv_bs, *fu_bs)]
    BIr = np.ascontiguousarray(np.tile(np.stack(bs, axis=1), (4, 1)), dtype=f)
    return Wk, Wsr, Wfold, Wf16, Wrep16, BIr


def kernel(E_a, E_s, E_r, V_a, u, fe_Ws, fe_bs, fv_Ws, fv_bs, fu_Ws, fu_bs,
           _want_trace=False):
    from concourse import bass_utils

    f = np.float32
    hp = np.float16
    E_a = np.asarray(E_a, f)
    E_s = np.asarray(E_s, f)
    E_r = np.asarray(E_r, f)
    V_a = np.ascontiguousarray(np.asarray(V_a, f))
    u = np.ascontiguousarray(np.asarray(u, f))
    import ml_dtypes
    fp8 = ml_dtypes.float8_e4m3
    # partition-major relayouts: [B, 128p, chunks, free]
    E_s16 = np.ascontiguousarray(
        E_s.astype(hp).reshape(B, VC, 128, NE).transpose(0, 2, 1, 3))
    E_r16 = np.ascontiguousarray(
        E_r.astype(hp).reshape(B, VC, 128, NE).transpose(0, 2, 1, 3))
    E_rT8 = np.ascontiguousarray(
        E_r.transpose(0, 2, 1).astype(fp8)
        .reshape(B, EC, 128, NV).transpose(0, 2, 1, 3))
    E_a16 = np.ascontiguousarray(E_a.astype(hp))
    V_a16 = np.ascontiguousarray(V_a.astype(hp))
    Wk, Wsr, Wfold, Wf16, Wrep16, BIr = _pack_host(fe_Ws, fe_bs, fv_Ws, fv_bs,
                                                   fu_Ws, fu_bs)

    nc = _get_nc()
    in_maps = []
    for i in range(N_CORES):
        sl = slice(i * GPC, (i + 1) * GPC)
        uT = np.ascontiguousarray(u[sl].T)
        in_maps.append({
            "E_s16": E_s16[sl], "E_r16": E_r16[sl], "E_rT8": E_rT8[sl],
            "E_a16": E_a16[sl], "V_a": V_a[sl], "V_a16": V_a16[sl],
            "uT": uT, "u16": uT.astype(hp),
            "Wk": Wk, "Wsr": Wsr, "Wfold": Wfold, "Wf16": Wf16,
            "Wrep16": Wrep16, "BIr": BIr,
        })
    res = bass_utils.run_bass_kernel_spmd(
        nc, in_maps, core_ids=list(range(N_CORES)), trace=_want_trace)

    out_E = np.concatenate([res.results[i]["out_E"] for i in range(N_CORES)], 0)
    out_V = np.concatenate([res.results[i]["out_V"] for i in range(N_CORES)], 0)
    out_u = np.concatenate([res.results[i]["out_uT"].T for i in range(N_CORES)],
                           0)
    out = (out_E.astype(f), out_V.astype(f), out_u.astype(f))
    if _want_trace:
        _cached["last_result"] = res
    return out


# revision 30
# speedup vs baseline: 1.1670x; 1.1670x over previous
"""Trainium2 Bass kernel for nn_A2CDense (dense GNN message-passing block).

Data-parallel over the graph-batch dim B=64: 8 graphs per NeuronCore, 8 cores.

Structure (per core, 8 graphs):
- The big one-hot incidence operands stream from HBM in fp16 (one-hot
  matrices are exact in fp16): E_s in [v,e] layout, E_r in BOTH [v,e] and
  [e,v] layouts (so the scatter-aggregate needs no on-chip transpose of the
  2048-wide matrix).
- Edge-MLP layer 1 uses associativity through the one-hot gathers:
    h1 = W1_Ea @ E_a + (W1_snd @ V_a) @ E_s + (W1_rcv @ V_a) @ E_r
  with the broadcast global-feature term folded into the activation bias.
  The four 512-edge tiles of a graph are STACKED across the 4 PE
  partition-quadrants via tile_position column-packing, so the four tiles'
  matmuls run concurrently on the 16 32x32 sub-arrays and each MLP layer
  needs ONE 128-partition gelu instead of four.
- Layers 2-3 run diagonal-packed (row=col=32j); layer 4 row-pack-unstacks
  back to flat [32, 2048] so out_E, its transposes (for the aggregate), and
  the per-graph reductions stay simple.
- agg = out_E @ E_r^T accumulates column-packed into a stacked [128, 512]
  PSUM (chunk k -> quadrant k%4); the node-MLP layer-1 aggregate term uses
  4x-replicated weights so the quadrant partial sums collapse in the same
  matmul. Node MLPs of 4 graphs run stacked across quadrants as one wave.
- PE matmul dtype is float32r (tf32-like, full-rate) for the small precise
  paths (M_s/M_r, folds, global MLP, out_E transposes); fp16 for the big
  streamed paths. PSUM accumulation is always fp32.
"""
import numpy as np

B, NV, NE = 64, 512, 2048
DF = 32
N_CORES = 8
GPC = B // N_CORES  # graphs per core
ET = 4              # edge tiles per graph
ETW = NE // ET      # 512
VC = NV // 128      # 4 v-chunks
EC = NE // 128      # 16 e-chunks
NW = GPC // 4       # node waves per core

_cached = {}

# Wk (f32r) indices
FE_U, FV_U, FU_U, FU_VS, FU_ES, FU_W2, FU_W3, FU_W4 = range(8)
# Wf16 indices
F16_EA, F16_VA, F16_VU = 0, 1, 2
# Wrep16 indices (all 4x vertically replicated)
R_FEW2, R_FEW3, R_FEW4, R_FVW2, R_FVW3, R_FVW4, R_AGG = range(7)


def _build_nc():
    import concourse.bass as bass
    import concourse.bacc as bacc
    import concourse.tile as tile
    import concourse.mybir as mybir
    from concourse.masks import make_identity

    f32 = mybir.dt.float32
    f32r = mybir.dt.float32r
    fp16 = mybir.dt.float16
    fp8 = mybir.dt.float8e4
    AF = mybir.ActivationFunctionType

    nc = bacc.Bacc("TRN2", target_bir_lowering=False, debug=False,
                   num_devices=N_CORES)

    E_s8 = nc.dram_tensor("E_s8", [GPC, 128, VC, NE], fp8, kind="ExternalInput")
    E_r8 = nc.dram_tensor("E_r8", [GPC, 128, VC, NE], fp8, kind="ExternalInput")
    E_rT8 = nc.dram_tensor("E_rT8", [GPC, 128, EC, NV], fp8,
                           kind="ExternalInput")
    E_a16 = nc.dram_tensor("E_a16", [GPC, DF, NE], fp16, kind="ExternalInput")
    V_a = nc.dram_tensor("V_a", [GPC, DF, NV], f32r, kind="ExternalInput")
    V_a16 = nc.dram_tensor("V_a16", [GPC, DF, NV], fp16, kind="ExternalInput")
    uT = nc.dram_tensor("uT", [DF, GPC], f32r, kind="ExternalInput")
    u16 = nc.dram_tensor("u16", [DF, GPC], fp16, kind="ExternalInput")
    Wk = nc.dram_tensor("Wk", [DF, 8, DF], f32r, kind="ExternalInput")
    Wsr = nc.dram_tensor("Wsr", [DF, 2 * DF], f32r, kind="ExternalInput")
    Wfold = nc.dram_tensor("Wfold", [DF, 128], f32r, kind="ExternalInput")
    Wf16 = nc.dram_tensor("Wf16", [DF, 3, DF], fp16, kind="ExternalInput")
    Wrep16 = nc.dram_tensor("Wrep16", [128, 7, DF], fp16,
                            kind="ExternalInput")
    BIr = nc.dram_tensor("BIr", [128, 12], f32, kind="ExternalInput")

    out_E = nc.dram_tensor("out_E", [GPC, DF, NE], f32r, kind="ExternalOutput")
    out_V = nc.dram_tensor("out_V", [GPC, DF, NV], f32r, kind="ExternalOutput")
    out_uT = nc.dram_tensor("out_uT", [DF, GPC], f32r, kind="ExternalOutput")

    with tile.TileContext(nc) as tc:
        with (
            tc.tile_pool(name="consts", bufs=1) as consts,
            tc.tile_pool(name="work", bufs=1) as work,
        ):
            W = consts.tile([DF, 8, DF], f32r, tag="W")
            Wsr_t = consts.tile([DF, 2 * DF], f32r, tag="Wsr")
            Wfold_t = consts.tile([DF, 128], f32r, tag="Wfold")
            Wf = consts.tile([DF, 3, DF], fp16, tag="Wf")
            Wr = consts.tile([128, 7, DF], fp16, tag="Wr")
            BIt = consts.tile([128, 12], f32, tag="BI")
            u_all = consts.tile([DF, GPC], f32r, tag="u_all")
            u16_t = consts.tile([DF, GPC], fp16, tag="u16")
            ident = consts.tile([128, 128], f32, tag="ident")
            ident_r = consts.tile([128, 128], f32r, tag="ident_r")
            for t, src in ((W, Wk), (Wsr_t, Wsr), (Wfold_t, Wfold), (Wf, Wf16),
                           (Wr, Wrep16), (BIt, BIr), (u_all, uT), (u16_t, u16)):
                nc.sync.dma_start(t[:], src.ap())
            make_identity(nc, ident[:])
            nc.vector.tensor_copy(ident_r[:], ident[:])

            # folded L1 biases
            b1e_st = work.tile([128, GPC], f32, tag="b1e")   # per graph
            b1v_st = work.tile([128, NW], f32, tag="b1v")    # per node wave
            with tc.tile_pool(name="pfold", bufs=1, space="PSUM") as pfold:
                ps_be = pfold.tile([128, GPC], f32, tag="pbe")
                nc.tensor.matmul(ps_be[:], Wfold_t[:], u_all[:],
                                 start=True, stop=True)
                nc.vector.tensor_scalar_add(b1e_st[:], ps_be[:], BIt[:, 0:1])
                ps_bv = pfold.tile([128, NW], f32, tag="pbv")
                for gi in range(4):
                    nc.tensor.matmul(
                        ps_bv[32 * gi:32 * (gi + 1), :], Wf[:, F16_VU, :],
                        u16_t[:].rearrange("d (w gi) -> d w gi", gi=4)
                        [:, :, gi],
                        start=True, stop=True, tile_position=(0, 32 * gi),
                        skip_group_check=True)
                nc.vector.tensor_scalar_add(b1v_st[:], ps_bv[:], BIt[:, 4:5])

            oVsum = work.tile([DF, GPC], f32, tag="oVsum")
            oEsum = work.tile([DF, GPC], f32, tag="oEsum")

            with (
                tc.tile_pool(name="big", bufs=2) as big,
                tc.tile_pool(name="big3", bufs=3) as big3,
                tc.tile_pool(name="med", bufs=2) as med,
                tc.tile_pool(name="node", bufs=6) as nodep,
                tc.tile_pool(name="small", bufs=3) as small,
                tc.tile_pool(name="pst", bufs=2, space="PSUM") as pstp,
                tc.tile_pool(name="pagg", bufs=2, space="PSUM") as paggp,
                tc.tile_pool(name="ph4", bufs=1, space="PSUM") as ph4p,
                tc.tile_pool(name="poet", bufs=2, space="PSUM") as poetp,
            ):
                lanes = [None] * 4  # (g, va16_t, agg16_t) per node lane

                def p0_load(g, st):
                    st["va"] = med.tile([DF, NV], f32r, tag="va")
                    st["va16"] = nodep.tile([DF, NV], fp16, tag="va16")
                    st["ea"] = med.tile([DF, NE], fp16, tag="ea")
                    nc.sync.dma_start(st["va"][:], V_a.ap()[g])
                    nc.sync.dma_start(st["va16"][:], V_a16.ap()[g])
                    nc.sync.dma_start(st["ea"][:], E_a16.ap()[g])
                    st["es"] = big.tile([128, VC, NE], fp16, tag="es")
                    st["er"] = big.tile([128, VC, NE], fp16, tag="er")
                    st["ert"] = big3.tile([128, EC, NV], fp8, tag="ert")
                    for hh in range(2):
                        sl = slice(hh * (NE // 2), (hh + 1) * (NE // 2))
                        nc.sync.dma_start(st["es"][:, :, sl],
                                          E_s8.ap()[g][:, :, sl])
                        nc.sync.dma_start(st["er"][:, :, sl],
                                          E_r8.ap()[g][:, :, sl])
                        kl = slice(hh * (EC // 2), (hh + 1) * (EC // 2))
                        nc.gpsimd.dma_start(st["ert"][:, kl, :],
                                            E_rT8.ap()[g][:, kl, :])
                    ps_m = paggp.tile([128, VC, 4 * DF], f32, tag="pagg")
                    for c in range(VC):
                        nc.tensor.matmul(ps_m[:, c, 0:2 * DF],
                                         st["va"][:, c * 128:(c + 1) * 128],
                                         Wsr_t[:], start=True, stop=True)
                    st["msrT"] = med.tile([128, VC, 2 * DF], fp16, tag="msrT")
                    nc.vector.tensor_copy(st["msrT"][:], ps_m[:, :, 0:2 * DF])

                def p1_l1(g, st):
                    msrT = st["msrT"]
                    pst = pstp.tile([128, ETW], f32, tag="pst")
                    for j in range(ET):
                        nc.tensor.matmul(
                            pst[32 * j:32 * (j + 1), :], Wf[:, F16_EA, :],
                            st["ea"][:, j * ETW:(j + 1) * ETW],
                            start=True, stop=False, tile_position=(0, 32 * j),
                            skip_group_check=True)
                    for mt in (msrT, st["msrTl"]):
                        for c in range(VC):
                            for j in range(ET):
                                nc.tensor.matmul(
                                    pst[32 * j:32 * (j + 1), :], mt[:, c, 0:DF],
                                    st["es"][:, c, j * ETW:(j + 1) * ETW],
                                    start=False, stop=False,
                                    tile_position=(0, 32 * j),
                                    skip_group_check=True)
                    for mi, mt in enumerate((msrT, st["msrTl"])):
                        for c in range(VC):
                            for j in range(ET):
                                nc.tensor.matmul(
                                    pst[32 * j:32 * (j + 1), :],
                                    mt[:, c, DF:2 * DF],
                                    st["er"][:, c, j * ETW:(j + 1) * ETW],
                                    start=False,
                                    stop=(mi == 1 and c == VC - 1),
                                    tile_position=(0, 32 * j),
                                    skip_group_check=True)
                    st["h"] = small.tile([128, ETW], fp16, tag="h")
                    nc.scalar.activation(st["h"][:], pst[:], AF.Gelu,
                                         bias=b1e_st[:, g:g + 1])

                def mk_l23(wi, bi):
                    def ph(g, st):
                        pst2 = pstp.tile([128, ETW], f32, tag="pst")
                        for j in range(ET):
                            nc.tensor.matmul(
                                pst2[32 * j:32 * (j + 1), :],
                                Wr[32 * j:32 * (j + 1), wi, :],
                                st["h"][32 * j:32 * (j + 1), :],
                                start=True, stop=True,
                                tile_position=(32 * j, 32 * j),
                                skip_group_check=True)
                        st["h"] = small.tile([128, ETW], fp16, tag="h")
                        nc.scalar.activation(st["h"][:], pst2[:], AF.Gelu,
                                             bias=BIt[:, bi:bi + 1])
                    return ph

                p2_l2 = mk_l23(R_FEW2, 1)
                p3_l3 = mk_l23(R_FEW3, 2)

                def p4_l4(g, st):
                    st["oe"] = med.tile([DF, NE], f32r, tag="oe")
                    st["oeacc"] = small.tile([DF, ET], f32, tag="oeacc")
                    for half in range(2):
                        ph4 = ph4p.tile([DF, 2, ETW], f32, tag="ph4")
                        for jj in range(2):
                            j = half * 2 + jj
                            nc.tensor.matmul(
                                ph4[:, jj, :],
                                Wr[32 * j:32 * (j + 1), R_FEW4, :],
                                st["h"][32 * j:32 * (j + 1), :],
                                start=True, stop=True,
                                tile_position=(32 * j, 0),
                                skip_group_check=True)
                        for jj in range(2):
                            et = half * 2 + jj
                            nc.scalar.activation(
                                st["oe"][:, et * ETW:(et + 1) * ETW],
                                ph4[:, jj, :], AF.Gelu, bias=BIt[0:DF, 3:4],
                                accum_out=st["oeacc"][:, et:et + 1])

                def p5_agg(g, st, ets):
                    for et in ets:
                        if "pagg" not in st:
                            st["pagg"] = paggp.tile([128, NV], f32,
                                                    tag="pagg", name="pagg")
                        pt = poetp.tile([128, VC, DF], f32r, tag="poet")
                        for c in range(VC):
                            nc.tensor.transpose(
                                pt[:, c, :],
                                st["oe"][:, et * ETW + c * 128:
                                         et * ETW + (c + 1) * 128],
                                ident_r[0:DF, 0:DF])
                        oeT = small.tile([128, VC, DF], fp8, tag="oeT")
                        oeTl = small.tile([128, VC, DF], fp8, tag="oeTl")
                        nc.vector.tensor_copy(oeT[:], pt[:])
                        nc.vector.tensor_sub(oeTl[:], pt[:], oeT[:])
                        for c in range(VC):
                            k = et * VC + c
                            j = k % 4
                            nc.tensor.matmul(
                                st["pagg"][32 * j:32 * (j + 1), :],
                                oeT[:, c, :], st["ert"][:, k, :],
                                start=(k < 4), stop=False,
                                tile_position=(0, 32 * j),
                                skip_group_check=True)
                            nc.tensor.matmul(
                                st["pagg"][32 * j:32 * (j + 1), :],
                                oeTl[:, c, :], st["ert"][:, k, :],
                                start=False, stop=(k >= EC - 4),
                                tile_position=(0, 32 * j),
                                skip_group_check=True)

                def p6_fin(g, st):
                    nc.sync.dma_start(out_E.ap()[g], st["oe"][:])
                    nc.vector.reduce_sum(oEsum[:, g:g + 1], st["oeacc"][:],
                                         axis=mybir.AxisListType.X)
                    agg16 = nodep.tile([128, NV], fp16, tag="agg16")
                    nc.vector.tensor_copy(agg16[:], st["pagg"][:])
                    lanes[g % 4] = (g, st["va16"], agg16)

                def make_wave_phases(w):
                    lsnap = list(lanes)
                    stw = {}

                    def n1():
                        pvn = pstp.tile([128, NV], f32, tag="pst")
                        for gi in range(4):
                            nc.tensor.matmul(
                                pvn[32 * gi:32 * (gi + 1), :],
                                Wf[:, F16_VA, :], lsnap[gi][1][:],
                                start=True, stop=False,
                                tile_position=(0, 32 * gi),
                                skip_group_check=True)
                        for gi in range(4):
                            nc.tensor.matmul(
                                pvn[32 * gi:32 * (gi + 1), :], Wr[:, R_AGG, :],
                                lsnap[gi][2][:], start=False, stop=True,
                                tile_position=(0, 32 * gi),
                                skip_group_check=True)
                        stw["hn"] = small.tile([128, NV], fp16, tag="hn")
                        nc.scalar.activation(stw["hn"][:], pvn[:], AF.Gelu,
                                             bias=b1v_st[:, w:w + 1])

                    def mk_n23(wi, bi):
                        def ph():
                            pvn2 = pstp.tile([128, NV], f32, tag="pst")
                            for gi in range(4):
                                nc.tensor.matmul(
                                    pvn2[32 * gi:32 * (gi + 1), :],
                                    Wr[32 * gi:32 * (gi + 1), wi, :],
                                    stw["hn"][32 * gi:32 * (gi + 1), :],
                                    start=True, stop=True,
                                    tile_position=(32 * gi, 32 * gi),
                                    skip_group_check=True)
                            stw["hn"] = small.tile([128, NV], fp16, tag="hn")
                            nc.scalar.activation(stw["hn"][:], pvn2[:],
                                                 AF.Gelu,
                                                 bias=BIt[:, bi:bi + 1])
                        return ph

                    def n4():
                        for half in range(2):
                            pn4 = ph4p.tile([DF, 2, NV], f32, tag="ph4")
                            for jj in range(2):
                                gi = half * 2 + jj
                                nc.tensor.matmul(
                                    pn4[:, jj, :],
                                    Wr[32 * gi:32 * (gi + 1), R_FVW4, :],
                                    stw["hn"][32 * gi:32 * (gi + 1), :],
                                    start=True, stop=True,
                                    tile_position=(32 * gi, 0),
                                    skip_group_check=True)
                            for jj in range(2):
                                gi = half * 2 + jj
                                g = lsnap[gi][0]
                                ov_t = nodep.tile([DF, NV], f32r, tag="ov")
                                nc.scalar.activation(
                                    ov_t[:], pn4[:, jj, :], AF.Gelu,
                                    bias=BIt[0:DF, 7:8],
                                    accum_out=oVsum[:, g:g + 1])
                                nc.sync.dma_start(out_V.ap()[g], ov_t[:])

                    return [n1, mk_n23(R_FVW2, 5), mk_n23(R_FVW3, 6), n4]

                wave_phases = []

                def take_wave():
                    if wave_phases:
                        wave_phases.pop(0)()

                sts = {}
                for pr in range(GPC // 2):
                    g0, g1 = 2 * pr, 2 * pr + 1
                    sts[g0], sts[g1] = {}, {}
                    p0_load(g0, sts[g0])
                    p0_load(g1, sts[g1])
                    p1_l1(g0, sts[g0])
                    take_wave()
                    p1_l1(g1, sts[g1])
                    take_wave()
                    p2_l2(g0, sts[g0])
                    p2_l2(g1, sts[g1])
                    take_wave()
                    p3_l3(g0, sts[g0])
                    p3_l3(g1, sts[g1])
                    take_wave()
                    p4_l4(g0, sts[g0])
                    p4_l4(g1, sts[g1])
                    p5_agg(g0, sts[g0], [0, 1])
                    p5_agg(g1, sts[g1], [0, 1])
                    p5_agg(g0, sts[g0], [2, 3])
                    p5_agg(g1, sts[g1], [2, 3])
                    p6_fin(g0, sts[g0])
                    p6_fin(g1, sts[g1])
                    del sts[g0], sts[g1]
                    while wave_phases:
                        wave_phases.pop(0)()
                    if pr % 2 == 1:
                        wave_phases = make_wave_phases(pr // 2)
                while wave_phases:
                    wave_phases.pop(0)()

                # --- global MLP: P = [u; sumV; sumE] ---
                vs_r = work.tile([DF, GPC], f32r, tag="vs_r")
                es_r = work.tile([DF, GPC], f32r, tag="es_r")
                nc.vector.tensor_copy(vs_r[:], oVsum[:])
                nc.vector.tensor_copy(es_r[:], oEsum[:])
                phu = pstp.tile([DF, GPC], f32, tag="pst")
                nc.tensor.matmul(phu[:], W[:, FU_U, :], u_all[:],
                                 start=True, stop=False)
                nc.tensor.matmul(phu[:], W[:, FU_VS, :], vs_r[:],
                                 start=False, stop=False)
                nc.tensor.matmul(phu[:], W[:, FU_ES, :], es_r[:],
                                 start=False, stop=True)
                hu = work.tile([DF, GPC], f32r, tag="hu")
                nc.scalar.activation(hu[:], phu[:], AF.Gelu,
                                     bias=BIt[0:DF, 8:9])
                for wi, bi in ((FU_W2, 9), (FU_W3, 10)):
                    phu2 = pstp.tile([DF, GPC], f32, tag="pst")
                    nc.tensor.matmul(phu2[:], W[:, wi, :], hu[:],
                                     start=True, stop=True)
                    hu = work.tile([DF, GPC], f32r, tag="hu")
                    nc.scalar.activation(hu[:], phu2[:], AF.Gelu,
                                         bias=BIt[0:DF, bi:bi + 1])
                phu4 = pstp.tile([DF, GPC], f32, tag="pst")
                nc.tensor.matmul(phu4[:], W[:, FU_W4, :], hu[:],
                                 start=True, stop=True)
                ou_t = work.tile([DF, GPC], f32r, tag="ou")
                nc.scalar.activation(ou_t[:], phu4[:], AF.Gelu,
                                     bias=BIt[0:DF, 11:12])
                nc.sync.dma_start(out_uT.ap(), ou_t[:])

    nc.compile()
    return nc


def _get_nc():
    if "nc" not in _cached:
        _cached["nc"] = _build_nc()
    return _cached["nc"]


def _pack_host(fe_Ws, fe_bs, fv_Ws, fv_bs, fu_Ws, fu_bs):
    import ml_dtypes
    f = np.float32
    hp = ml_dtypes.bfloat16  # placeholder, replaced below
    hp = np.float16
    fe_Ws = [np.asarray(w, f) for w in fe_Ws]
    fv_Ws = [np.asarray(w, f) for w in fv_Ws]
    fu_Ws = [np.asarray(w, f) for w in fu_Ws]
    W1e, W1v, W1u = fe_Ws[0], fv_Ws[0], fu_Ws[0]
    # f32r pack
    wk = [W1e[:, 96:128].T, W1v[:, 64:96].T,
          W1u[:, 0:32].T, W1u[:, 32:64].T, W1u[:, 64:96].T,
          fu_Ws[1].T, fu_Ws[2].T, fu_Ws[3].T]
    Wk = np.ascontiguousarray(np.stack(wk, axis=1), dtype=f)      # [32, 8, 32]
    Wsr = np.ascontiguousarray(
        np.concatenate([W1e[:, 32:64].T, W1e[:, 64:96].T], axis=1), dtype=f)
    Wfold = np.ascontiguousarray(np.tile(W1e[:, 96:128].T, (1, 4)), dtype=f)
    wf = [W1e[:, 0:32].T, W1v[:, 0:32].T, W1v[:, 64:96].T]
    Wf16 = np.ascontiguousarray(np.stack(wf, axis=1), dtype=hp)   # [32, 3, 32]
    wr = [fe_Ws[1].T, fe_Ws[2].T, fe_Ws[3].T,
          fv_Ws[1].T, fv_Ws[2].T, fv_Ws[3].T, W1v[:, 32:64].T]
    Wrep16 = np.ascontiguousarray(
        np.stack([np.tile(m, (4, 1)) for m in wr], axis=1), dtype=hp)
    bs = [np.asarray(b, f) for b in (*fe_bs, *fv_bs, *fu_bs)]
    BIr = np.ascontiguousarray(np.tile(np.stack(bs, axis=1), (4, 1)), dtype=f)
    return Wk, Wsr, Wfold, Wf16, Wrep16, BIr


def kernel(E_a, E_s, E_r, V_a, u, fe_Ws, fe_bs, fv_Ws, fv_bs, fu_Ws, fu_bs,
           _want_trace=False):
    from concourse import bass_utils

    f = np.float32
    hp = np.float16
    E_a = np.asarray(E_a, f)
    E_s = np.asarray(E_s, f)
    E_r = np.asarray(E_r, f)
    V_a = np.ascontiguousarray(np.asarray(V_a, f))
    u = np.ascontiguousarray(np.asarray(u, f))
    import ml_dtypes
    fp8 = ml_dtypes.float8_e4m3
    # partition-major relayouts: [B, 128p, chunks, free]
    E_s8 = np.ascontiguousarray(
        E_s.astype(fp8).reshape(B, VC, 128, NE).transpose(0, 2, 1, 3))
    E_r8 = np.ascontiguousarray(
        E_r.astype(fp8).reshape(B, VC, 128, NE).transpose(0, 2, 1, 3))
    E_rT8 = np.ascontiguousarray(
        E_r.transpose(0, 2, 1).astype(fp8)
        .reshape(B, EC, 128, NV).transpose(0, 2, 1, 3))
    E_a16 = np.ascontiguousarray(E_a.astype(hp))
    V_a16 = np.ascontiguousarray(V_a.astype(hp))
    Wk, Wsr, Wfold, Wf16, Wrep16, BIr = _pack_host(fe_Ws, fe_bs, fv_Ws, fv_bs,
                                                   fu_Ws, fu_bs)

    nc = _get_nc()
    in_maps = []
    for i in range(N_CORES):
        sl = slice(i * GPC, (i + 1) * GPC)
        uT = np.ascontiguousarray(u[sl].T)
        in_maps.append({
            "E_s8": E_s8[sl], "E_r8": E_r8[sl], "E_rT8": E_rT8[sl],
            "E_a16": E_a16[sl], "V_a": V_a[sl], "V_a16": V_a16[sl],
            "uT": uT, "u16": uT.astype(hp),
            "Wk": Wk, "Wsr": Wsr, "Wfold": Wfold, "Wf16": Wf16,
            "Wrep16": Wrep16, "BIr": BIr,
        })
    res = bass_utils.run_bass_kernel_spmd(
        nc, in_maps, core_ids=list(range(N_CORES)), trace=_want_trace)

    out_E = np.concatenate([res.results[i]["out_E"] for i in range(N_CORES)], 0)
    out_V = np.concatenate([res.results[i]["out_V"] for i in range(N_CORES)], 0)
    out_u = np.concatenate([res.results[i]["out_uT"].T for i in range(N_CORES)],
                           0)
    out = (out_E.astype(f), out_V.astype(f), out_u.astype(f))
    if _want_trace:
        _cached["last_result"] = res
    return out


# revision 33
# speedup vs baseline: 1.1757x; 1.0075x over previous
"""Trainium2 Bass kernel for nn_A2CDense (dense GNN message-passing block).

Data-parallel over the graph-batch dim B=64: 8 graphs per NeuronCore, 8 cores.

Structure (per core, 8 graphs):
- The big one-hot incidence operands stream from HBM in fp8-e4m3 (one-hot
  matrices are exactly representable in fp8): E_s in [v,e] layout, E_r in
  BOTH [v,e] and [e,v] layouts, so the scatter-aggregate needs no on-chip
  transpose of the 2048-wide matrix. Total HBM traffic is ~28 MB/core.
- Edge-MLP layer 1 uses associativity through the one-hot gathers:
    h1 = W1_Ea @ E_a + (W1_snd @ V_a) @ E_s + (W1_rcv @ V_a) @ E_r
  with the broadcast global-feature term folded into the activation bias.
  The dense [32 x 32] factors M = W1_x @ V_a are split hi+lo in fp8 and the
  two products accumulate in fp32 PSUM, recovering ~bf16 precision while
  streaming fp8. The same hi+lo split is applied to out_E^T for the
  aggregate matmuls.
- The four 512-edge tiles of a graph are STACKED across the 4 PE
  partition-quadrants via tile_position column-packing, so the four tiles'
  matmuls run concurrently on the 16 32x32 sub-arrays and each MLP layer
  needs ONE 128-partition gelu instead of four. Layers 2-3 run
  diagonal-packed (row=col=32j); layer 4 row-pack-unstacks back to flat
  [32, 2048] so out_E, its transposes, and per-graph reductions stay simple.
- agg = out_E @ E_r^T accumulates column-packed into a stacked [128, 512]
  PSUM (chunk k -> quadrant k%4); the node-MLP layer-1 aggregate term uses
  4x-replicated weights so the quadrant partial sums collapse in the same
  matmul. Node MLPs of 4 graphs run stacked across quadrants as one wave.
- Emission is software-pipelined two-graphs-at-a-time (ping-pong) for the
  in-order PE queue: every matmul->gelu dependency of graph A is covered by
  the twin wave of graph B; deferred node-wave phases splice into the next
  pair. float32r (tf32-like, full-rate) is used for the small precise paths
  (M_s/M_r, bias folds, global MLP, out_E transposes); PSUM accumulation is
  always fp32. Final rel err vs the fp32 reference is ~7e-4.
"""
import numpy as np

B, NV, NE = 64, 512, 2048
DF = 32
N_CORES = 8
GPC = B // N_CORES  # graphs per core
ET = 4              # edge tiles per graph
ETW = NE // ET      # 512
VC = NV // 128      # 4 v-chunks
EC = NE // 128      # 16 e-chunks
NW = GPC // 4       # node waves per core

_cached = {}

# Wk (f32r) indices
FE_U, FV_U, FU_U, FU_VS, FU_ES, FU_W2, FU_W3, FU_W4 = range(8)
# Wf16 indices
F16_EA, F16_VA, F16_VU = 0, 1, 2
# Wrep16 indices (all 4x vertically replicated)
R_FEW2, R_FEW3, R_FEW4, R_FVW2, R_FVW3, R_FVW4, R_AGG = range(7)


def _build_nc():
    import concourse.bass as bass
    import concourse.bacc as bacc
    import concourse.tile as tile
    import concourse.mybir as mybir
    from concourse.masks import make_identity

    f32 = mybir.dt.float32
    f32r = mybir.dt.float32r
    fp16 = mybir.dt.float16
    fp8 = mybir.dt.float8e4
    AF = mybir.ActivationFunctionType

    nc = bacc.Bacc("TRN2", target_bir_lowering=False, debug=False,
                   num_devices=N_CORES)

    E_s8 = nc.dram_tensor("E_s8", [GPC, 128, VC, NE], fp8, kind="ExternalInput")
    E_r8 = nc.dram_tensor("E_r8", [GPC, 128, VC, NE], fp8, kind="ExternalInput")
    E_rT8 = nc.dram_tensor("E_rT8", [GPC, 128, EC, NV], fp8,
                           kind="ExternalInput")
    E_a16 = nc.dram_tensor("E_a16", [GPC, DF, NE], fp16, kind="ExternalInput")
    V_a = nc.dram_tensor("V_a", [GPC, DF, NV], f32r, kind="ExternalInput")
    V_a16 = nc.dram_tensor("V_a16", [GPC, DF, NV], fp16, kind="ExternalInput")
    uT = nc.dram_tensor("uT", [DF, GPC], f32r, kind="ExternalInput")
    u16 = nc.dram_tensor("u16", [DF, GPC], fp16, kind="ExternalInput")
    Wk = nc.dram_tensor("Wk", [DF, 8, DF], f32r, kind="ExternalInput")
    Wsr = nc.dram_tensor("Wsr", [DF, 2 * DF], f32r, kind="ExternalInput")
    Wfold = nc.dram_tensor("Wfold", [DF, 128], f32r, kind="ExternalInput")
    Wf16 = nc.dram_tensor("Wf16", [DF, 3, DF], fp16, kind="ExternalInput")
    Wrep16 = nc.dram_tensor("Wrep16", [128, 7, DF], fp16,
                            kind="ExternalInput")
    BIr = nc.dram_tensor("BIr", [128, 12], f32, kind="ExternalInput")

    out_E = nc.dram_tensor("out_E", [GPC, DF, NE], f32r, kind="ExternalOutput")
    out_V = nc.dram_tensor("out_V", [GPC, DF, NV], f32r, kind="ExternalOutput")
    out_uT = nc.dram_tensor("out_uT", [DF, GPC], f32r, kind="ExternalOutput")

    with tile.TileContext(nc) as tc:
        with (
            tc.tile_pool(name="consts", bufs=1) as consts,
            tc.tile_pool(name="work", bufs=1) as work,
        ):
            W = consts.tile([DF, 8, DF], f32r, tag="W")
            Wsr_t = consts.tile([DF, 2 * DF], f32r, tag="Wsr")
            Wfold_t = consts.tile([DF, 128], f32r, tag="Wfold")
            Wf = consts.tile([DF, 3, DF], fp16, tag="Wf")
            Wr = consts.tile([128, 7, DF], fp16, tag="Wr")
            BIt = consts.tile([128, 12], f32, tag="BI")
            u_all = consts.tile([DF, GPC], f32r, tag="u_all")
            u16_t = consts.tile([DF, GPC], fp16, tag="u16")
            ident = consts.tile([128, 128], f32, tag="ident")
            ident_r = consts.tile([128, 128], f32r, tag="ident_r")
            for t, src in ((W, Wk), (Wsr_t, Wsr), (Wfold_t, Wfold), (Wf, Wf16),
                           (Wr, Wrep16), (BIt, BIr), (u_all, uT), (u16_t, u16)):
                nc.sync.dma_start(t[:], src.ap())
            make_identity(nc, ident[:])
            nc.vector.tensor_copy(ident_r[:], ident[:])

            # folded L1 biases
            b1e_st = work.tile([128, GPC], f32, tag="b1e")   # per graph
            b1v_st = work.tile([128, NW], f32, tag="b1v")    # per node wave
            with tc.tile_pool(name="pfold", bufs=1, space="PSUM") as pfold:
                ps_be = pfold.tile([128, GPC], f32, tag="pbe")
                nc.tensor.matmul(ps_be[:], Wfold_t[:], u_all[:],
                                 start=True, stop=True)
                nc.vector.tensor_scalar_add(b1e_st[:], ps_be[:], BIt[:, 0:1])
                ps_bv = pfold.tile([128, NW], f32, tag="pbv")
                for gi in range(4):
                    nc.tensor.matmul(
                        ps_bv[32 * gi:32 * (gi + 1), :], Wf[:, F16_VU, :],
                        u16_t[:].rearrange("d (w gi) -> d w gi", gi=4)
                        [:, :, gi],
                        start=True, stop=True, tile_position=(0, 32 * gi),
                        skip_group_check=True)
                nc.vector.tensor_scalar_add(b1v_st[:], ps_bv[:], BIt[:, 4:5])

            oVsum = work.tile([DF, GPC], f32, tag="oVsum")
            oEsum = work.tile([DF, GPC], f32, tag="oEsum")

            with (
                tc.tile_pool(name="big", bufs=2) as big,
                tc.tile_pool(name="big3", bufs=3) as big3,
                tc.tile_pool(name="med", bufs=2) as med,
                tc.tile_pool(name="node", bufs=6) as nodep,
                tc.tile_pool(name="small", bufs=3) as small,
                tc.tile_pool(name="pst", bufs=2, space="PSUM") as pstp,
                tc.tile_pool(name="pagg", bufs=2, space="PSUM") as paggp,
                tc.tile_pool(name="ph4", bufs=1, space="PSUM") as ph4p,
                tc.tile_pool(name="poet", bufs=2, space="PSUM") as poetp,
            ):
                lanes = [None] * 4  # (g, va16_t, agg16_t) per node lane

                def p0_load(g, st):
                    st["va"] = med.tile([DF, NV], f32r, tag="va", name="va")
                    st["va16"] = nodep.tile([DF, NV], fp16, tag="va16", name="va16")
                    st["ea"] = med.tile([DF, NE], fp16, tag="ea", name="ea")
                    nc.sync.dma_start(st["va"][:], V_a.ap()[g])
                    nc.sync.dma_start(st["va16"][:], V_a16.ap()[g])
                    nc.sync.dma_start(st["ea"][:], E_a16.ap()[g])
                    st["es"] = big3.tile([128, VC, NE], fp8, tag="es", name="es")
                    st["er"] = big3.tile([128, VC, NE], fp8, tag="er", name="er")
                    st["ert"] = big3.tile([128, EC, NV], fp8, tag="ert", name="ert")
                    for hh in range(2):
                        cl = slice(hh * (VC // 2), (hh + 1) * (VC // 2))
                        nc.sync.dma_start(st["es"][:, cl, :],
                                          E_s8.ap()[g][:, cl, :])
                        nc.sync.dma_start(st["er"][:, cl, :],
                                          E_r8.ap()[g][:, cl, :])
                        kl = slice(hh * (EC // 2), (hh + 1) * (EC // 2))
                        nc.gpsimd.dma_start(st["ert"][:, kl, :],
                                            E_rT8.ap()[g][:, kl, :])
                    ps_m = paggp.tile([128, VC, 4 * DF], f32, tag="pagg")
                    for c in range(VC):
                        nc.tensor.matmul(ps_m[:, c, 0:2 * DF],
                                         st["va"][:, c * 128:(c + 1) * 128],
                                         Wsr_t[:], start=True, stop=True)
                    st["msrT"] = med.tile([128, VC, 2 * DF], fp8, tag="msrT", name="msrT")
                    st["msrTl"] = med.tile([128, VC, 2 * DF], fp8, tag="msrTl", name="msrTl")
                    nc.vector.tensor_copy(st["msrT"][:], ps_m[:, :, 0:2 * DF])
                    nc.vector.tensor_sub(st["msrTl"][:], ps_m[:, :, 0:2 * DF],
                                         st["msrT"][:])

                def p1_l1(g, st):
                    msrT = st["msrT"]
                    pst = pstp.tile([128, ETW], f32, tag="pst")
                    for j in range(ET):
                        nc.tensor.matmul(
                            pst[32 * j:32 * (j + 1), :], Wf[:, F16_EA, :],
                            st["ea"][:, j * ETW:(j + 1) * ETW],
                            start=True, stop=False, tile_position=(0, 32 * j),
                            skip_group_check=True)
                    for mt in (msrT, st["msrTl"]):
                        for c in range(VC):
                            for j in range(ET):
                                nc.tensor.matmul(
                                    pst[32 * j:32 * (j + 1), :], mt[:, c, 0:DF],
                                    st["es"][:, c, j * ETW:(j + 1) * ETW],
                                    start=False, stop=False,
                                    tile_position=(0, 32 * j),
                                    skip_group_check=True)
                    for mi, mt in enumerate((msrT, st["msrTl"])):
                        for c in range(VC):
                            for j in range(ET):
                                nc.tensor.matmul(
                                    pst[32 * j:32 * (j + 1), :],
                                    mt[:, c, DF:2 * DF],
                                    st["er"][:, c, j * ETW:(j + 1) * ETW],
                                    start=False,
                                    stop=(mi == 1 and c == VC - 1),
                                    tile_position=(0, 32 * j),
                                    skip_group_check=True)
                    st["h"] = small.tile([128, ETW], fp16, tag="h", name="h")
                    nc.scalar.activation(st["h"][:], pst[:], AF.Gelu,
                                         bias=b1e_st[:, g:g + 1])

                def mk_l23(wi, bi):
                    def ph(g, st):
                        pst2 = pstp.tile([128, ETW], f32, tag="pst")
                        for j in range(ET):
                            nc.tensor.matmul(
                                pst2[32 * j:32 * (j + 1), :],
                                Wr[32 * j:32 * (j + 1), wi, :],
                                st["h"][32 * j:32 * (j + 1), :],
                                start=True, stop=True,
                                tile_position=(32 * j, 32 * j),
                                skip_group_check=True)
                        st["h"] = small.tile([128, ETW], fp16, tag="h", name="h")
                        nc.scalar.activation(st["h"][:], pst2[:], AF.Gelu,
                                             bias=BIt[:, bi:bi + 1])
                    return ph

                p2_l2 = mk_l23(R_FEW2, 1)
                p3_l3 = mk_l23(R_FEW3, 2)

                def p4_l4(g, st):
                    st["oe"] = med.tile([DF, NE], f32r, tag="oe", name="oe")
                    st["oeacc"] = small.tile([DF, ET], f32, tag="oeacc", name="oeacc")
                    for half in range(2):
                        ph4 = ph4p.tile([DF, 2, ETW], f32, tag="ph4")
                        for jj in range(2):
                            j = half * 2 + jj
                            nc.tensor.matmul(
                                ph4[:, jj, :],
                                Wr[32 * j:32 * (j + 1), R_FEW4, :],
                                st["h"][32 * j:32 * (j + 1), :],
                                start=True, stop=True,
                                tile_position=(32 * j, 0),
                                skip_group_check=True)
                        for jj in range(2):
                            et = half * 2 + jj
                            nc.scalar.activation(
                                st["oe"][:, et * ETW:(et + 1) * ETW],
                                ph4[:, jj, :], AF.Gelu, bias=BIt[0:DF, 3:4],
                                accum_out=st["oeacc"][:, et:et + 1])

                def p5_agg(g, st, ets):
                    for et in ets:
                        if "pagg" not in st:
                            st["pagg"] = paggp.tile([128, NV], f32,
                                                    tag="pagg", name="pagg")
                        pt = poetp.tile([128, VC, DF], f32r, tag="poet")
                        for c in range(VC):
                            nc.tensor.transpose(
                                pt[:, c, :],
                                st["oe"][:, et * ETW + c * 128:
                                         et * ETW + (c + 1) * 128],
                                ident_r[0:DF, 0:DF])
                        oeT = small.tile([128, VC, DF], fp8, tag="oeT")
                        oeTl = small.tile([128, VC, DF], fp8, tag="oeTl")
                        nc.vector.tensor_copy(oeT[:], pt[:])
                        nc.vector.tensor_sub(oeTl[:], pt[:], oeT[:])
                        for c in range(VC):
                            k = et * VC + c
                            j = k % 4
                            nc.tensor.matmul(
                                st["pagg"][32 * j:32 * (j + 1), :],
                                oeT[:, c, :], st["ert"][:, k, :],
                                start=(k < 4), stop=False,
                                tile_position=(0, 32 * j),
                                skip_group_check=True)
                            nc.tensor.matmul(
                                st["pagg"][32 * j:32 * (j + 1), :],
                                oeTl[:, c, :], st["ert"][:, k, :],
                                start=False, stop=(k >= EC - 4),
                                tile_position=(0, 32 * j),
                                skip_group_check=True)

                def p6_fin(g, st):
                    nc.sync.dma_start(out_E.ap()[g], st["oe"][:])
                    nc.vector.reduce_sum(oEsum[:, g:g + 1], st["oeacc"][:],
                                         axis=mybir.AxisListType.X)
                    agg16 = nodep.tile([128, NV], fp16, tag="agg16")
                    nc.vector.tensor_copy(agg16[:], st["pagg"][:])
                    lanes[g % 4] = (g, st["va16"], agg16)

                def make_wave_phases(w):
                    lsnap = list(lanes)
                    stw = {}

                    def n1():
                        pvn = pstp.tile([128, NV], f32, tag="pst")
                        for gi in range(4):
                            nc.tensor.matmul(
                                pvn[32 * gi:32 * (gi + 1), :],
                                Wf[:, F16_VA, :], lsnap[gi][1][:],
                                start=True, stop=False,
                                tile_position=(0, 32 * gi),
                                skip_group_check=True)
                        for gi in range(4):
                            nc.tensor.matmul(
                                pvn[32 * gi:32 * (gi + 1), :], Wr[:, R_AGG, :],
                                lsnap[gi][2][:], start=False, stop=True,
                                tile_position=(0, 32 * gi),
                                skip_group_check=True)
                        stw["hn"] = small.tile([128, NV], fp16, tag="hn", name="hn")
                        nc.scalar.activation(stw["hn"][:], pvn[:], AF.Gelu,
                                             bias=b1v_st[:, w:w + 1])

                    def mk_n23(wi, bi):
                        def ph():
                            pvn2 = pstp.tile([128, NV], f32, tag="pst")
                            for gi in range(4):
                                nc.tensor.matmul(
                                    pvn2[32 * gi:32 * (gi + 1), :],
                                    Wr[32 * gi:32 * (gi + 1), wi, :],
                                    stw["hn"][32 * gi:32 * (gi + 1), :],
                                    start=True, stop=True,
                                    tile_position=(32 * gi, 32 * gi),
                                    skip_group_check=True)
                            stw["hn"] = small.tile([128, NV], fp16, tag="hn", name="hn")
                            nc.scalar.activation(stw["hn"][:], pvn2[:],
                                                 AF.Gelu,
                                                 bias=BIt[:, bi:bi + 1])
                        return ph

                    def n4():
                        for half in range(2):
                            pn4 = ph4p.tile([DF, 2, NV], f32, tag="ph4")
                            for jj in range(2):
                                gi = half * 2 + jj
                                nc.tensor.matmul(
                                    pn4[:, jj, :],
                                    Wr[32 * gi:32 * (gi + 1), R_FVW4, :],
                                    stw["hn"][32 * gi:32 * (gi + 1), :],
                                    start=True, stop=True,
                                    tile_position=(32 * gi, 0),
                                    skip_group_check=True)
                            for jj in range(2):
                                gi = half * 2 + jj
                                g = lsnap[gi][0]
                                ov_t = nodep.tile([DF, NV], f32r, tag="ov")
                                nc.scalar.activation(
                                    ov_t[:], pn4[:, jj, :], AF.Gelu,
                                    bias=BIt[0:DF, 7:8],
                                    accum_out=oVsum[:, g:g + 1])
                                nc.sync.dma_start(out_V.ap()[g], ov_t[:])

                    return [n1, mk_n23(R_FVW2, 5), mk_n23(R_FVW3, 6), n4]

                wave_phases = []

                def take_wave():
                    if wave_phases:
                        wave_phases.pop(0)()

                sts = {}
                for pr in range(GPC // 2):
                    g0, g1 = 2 * pr, 2 * pr + 1
                    sts[g0], sts[g1] = {}, {}
                    p0_load(g0, sts[g0])
                    p0_load(g1, sts[g1])
                    p1_l1(g0, sts[g0])
                    take_wave()
                    p1_l1(g1, sts[g1])
                    take_wave()
                    p2_l2(g0, sts[g0])
                    p2_l2(g1, sts[g1])
                    take_wave()
                    p3_l3(g0, sts[g0])
                    p3_l3(g1, sts[g1])
                    take_wave()
                    p4_l4(g0, sts[g0])
                    p4_l4(g1, sts[g1])
                    p5_agg(g0, sts[g0], [0, 1])
                    p5_agg(g1, sts[g1], [0, 1])
                    p5_agg(g0, sts[g0], [2, 3])
                    p5_agg(g1, sts[g1], [2, 3])
                    p6_fin(g0, sts[g0])
                    p6_fin(g1, sts[g1])
                    del sts[g0], sts[g1]
                    while wave_phases:
                        wave_phases.pop(0)()
                    if pr % 2 == 1:
                        wave_phases = make_wave_phases(pr // 2)
                while wave_phases:
                    wave_phases.pop(0)()

                # --- global MLP: P = [u; sumV; sumE] ---
                vs_r = work.tile([DF, GPC], f32r, tag="vs_r")
                es_r = work.tile([DF, GPC], f32r, tag="es_r")
                nc.vector.tensor_copy(vs_r[:], oVsum[:])
                nc.vector.tensor_copy(es_r[:], oEsum[:])
                phu = pstp.tile([DF, GPC], f32, tag="pst")
                nc.tensor.matmul(phu[:], W[:, FU_U, :], u_all[:],
                                 start=True, stop=False)
                nc.tensor.matmul(phu[:], W[:, FU_VS, :], vs_r[:],
                                 start=False, stop=False)
                nc.tensor.matmul(phu[:], W[:, FU_ES, :], es_r[:],
                                 start=False, stop=True)
                hu = work.tile([DF, GPC], f32r, tag="hu")
                nc.scalar.activation(hu[:], phu[:], AF.Gelu,
                                     bias=BIt[0:DF, 8:9])
                for wi, bi in ((FU_W2, 9), (FU_W3, 10)):
                    phu2 = pstp.tile([DF, GPC], f32, tag="pst")
                    nc.tensor.matmul(phu2[:], W[:, wi, :], hu[:],
                                     start=True, stop=True)
                    hu = work.tile([DF, GPC], f32r, tag="hu")
                    nc.scalar.activation(hu[:], phu2[:], AF.Gelu,
                                         bias=BIt[0:DF, bi:bi + 1])
                phu4 = pstp.tile([DF, GPC], f32, tag="pst")
                nc.tensor.matmul(phu4[:], W[:, FU_W4, :], hu[:],
                                 start=True, stop=True)
                ou_t = work.tile([DF, GPC], f32r, tag="ou")
                nc.scalar.activation(ou_t[:], phu4[:], AF.Gelu,
                                     bias=BIt[0:DF, 11:12])
                nc.sync.dma_start(out_uT.ap(), ou_t[:])

    nc.compile()
    return nc


def _get_nc():
    if "nc" not in _cached:
        _cached["nc"] = _build_nc()
    return _cached["nc"]


def _pack_host(fe_Ws, fe_bs, fv_Ws, fv_bs, fu_Ws, fu_bs):
    f = np.float32
    hp = np.float16
    fe_Ws = [np.asarray(w, f) for w in fe_Ws]
    fv_Ws = [np.asarray(w, f) for w in fv_Ws]
    fu_Ws = [np.asarray(w, f) for w in fu_Ws]
    W1e, W1v, W1u = fe_Ws[0], fv_Ws[0], fu_Ws[0]
    # f32r pack
    wk = [W1e[:, 96:128].T, W1v[:, 64:96].T,
          W1u[:, 0:32].T, W1u[:, 32:64].T, W1u[:, 64:96].T,
          fu_Ws[1].T, fu_Ws[2].T, fu_Ws[3].T]
    Wk = np.ascontiguousarray(np.stack(wk, axis=1), dtype=f)      # [32, 8, 32]
    Wsr = np.ascontiguousarray(
        np.concatenate([W1e[:, 32:64].T, W1e[:, 64:96].T], axis=1), dtype=f)
    Wfold = np.ascontiguousarray(np.tile(W1e[:, 96:128].T, (1, 4)), dtype=f)
    wf = [W1e[:, 0:32].T, W1v[:, 0:32].T, W1v[:, 64:96].T]
    Wf16 = np.ascontiguousarray(np.stack(wf, axis=1), dtype=hp)   # [32, 3, 32]
    wr = [fe_Ws[1].T, fe_Ws[2].T, fe_Ws[3].T,
          fv_Ws[1].T, fv_Ws[2].T, fv_Ws[3].T, W1v[:, 32:64].T]
    Wrep16 = np.ascontiguousarray(
        np.stack([np.tile(m, (4, 1)) for m in wr], axis=1), dtype=hp)
    bs = [np.asarray(b, f) for b in (*fe_bs, *fv_bs, *fu_bs)]
    BIr = np.ascontiguousarray(np.tile(np.stack(bs, axis=1), (4, 1)), dtype=f)
    return Wk, Wsr, Wfold, Wf16, Wrep16, BIr


def kernel(E_a, E_s, E_r, V_a, u, fe_Ws, fe_bs, fv_Ws, fv_bs, fu_Ws, fu_bs,
           _want_trace=False):
    from concourse import bass_utils

    f = np.float32
    hp = np.float16
    E_a = np.asarray(E_a, f)
    E_s = np.asarray(E_s, f)
    E_r = np.asarray(E_r, f)
    V_a = np.ascontiguousarray(np.asarray(V_a, f))
    u = np.ascontiguousarray(np.asarray(u, f))
    import ml_dtypes
    fp8 = ml_dtypes.float8_e4m3
    # partition-major relayouts: [B, 128p, chunks, free]
    E_s8 = np.ascontiguousarray(
        E_s.astype(fp8).reshape(B, VC, 128, NE).transpose(0, 2, 1, 3))
    E_r8 = np.ascontiguousarray(
        E_r.astype(fp8).reshape(B, VC, 128, NE).transpose(0, 2, 1, 3))
    E_rT8 = np.ascontiguousarray(
        E_r.transpose(0, 2, 1).astype(fp8)
        .reshape(B, EC, 128, NV).transpose(0, 2, 1, 3))
    E_a16 = np.ascontiguousarray(E_a.astype(hp))
    V_a16 = np.ascontiguousarray(V_a.astype(hp))
    Wk, Wsr, Wfold, Wf16, Wrep16, BIr = _pack_host(fe_Ws, fe_bs, fv_Ws, fv_bs,
                                                   fu_Ws, fu_bs)

    nc = _get_nc()
    in_maps = []
    for i in range(N_CORES):
        sl = slice(i * GPC, (i + 1) * GPC)
        uT = np.ascontiguousarray(u[sl].T)
        in_maps.append({
            "E_s8": E_s8[sl], "E_r8": E_r8[sl], "E_rT8": E_rT8[sl],
            "E_a16": E_a16[sl], "V_a": V_a[sl], "V_a16": V_a16[sl],
            "uT": uT, "u16": uT.astype(hp),
            "Wk": Wk, "Wsr": Wsr, "Wfold": Wfold, "Wf16": Wf16,
            "Wrep16": Wrep16, "BIr": BIr,
        })
    res = bass_utils.run_bass_kernel_spmd(
        nc, in_maps, core_ids=list(range(N_CORES)), trace=_want_trace)

    out_E = np.concatenate([res.results[i]["out_E"] for i in range(N_CORES)], 0)
    out_V = np.concatenate([res.results[i]["out_V"] for i in range(N_CORES)], 0)
    out_u = np.concatenate([res.results[i]["out_uT"].T for i in range(N_CORES)],
                           0)
    out = (out_E.astype(f), out_V.astype(f), out_u.astype(f))
    if _want_trace:
        _cached["last_result"] = res
    return out

